# revision 1
# baseline (speedup 1.0000x reference)
"""Masked cross-modal attention on 8 Trainium2 NeuronCores.

Reference math (per batch b):
    q,k,v = x @ W{q,k,v}.T   (head-major channels, H=8, Dh=64)
    s     = (q @ k.T) / 8, masked_fill(mask==0, 1e-9), softmax over keys
    out   = (att @ v) @ Wout.T

Masked positions contribute weight exp(1e-9)=1 and value v_j independent of
the query, so with U = unmasked keys, M = masked keys:
    out[t] = (sum_{j in U} e^{s_tj} v_j + sum_{j in M} v_j)
           / (sum_{j in U} e^{s_tj} + |M|)
The kernel runs attention only over gathered unmasked keys (~half) and the
masked-sum corrections are tiny host-side vectors added on-chip.

Sharding: core c -> batch c//2, head-group c%2 (4 of 8 heads). Each core
emits two partial [2048,512] outputs (one per head-pair through its Wout
slice); the host sums the four partials per batch.

Engine layout per core: PE does QKV projections (f32r), scores (bf16 Q^T/K^T,
row-pair packed), exp-weighted value sums with an indicator column for the
softmax denominator (f32r), and the output projection. ACT does all exp plus
the output-tile copies; DVE handles PSUM evacuation and the normalize chain.
QKV work for the second head-pair and Wout work for the first are drip-fed
between score/exp groups to keep ACT (the bottleneck) saturated.
"""

import sys

for _p in ("/opt/trn_rl_repo", "/root/.axon_site/_ro/trn_rl_repo"):
    if _p not in sys.path:
        sys.path.append(_p)

import numpy as np
import ml_dtypes
import concourse.bass as bass
import concourse.mybir as mybir
import concourse.tile as tile
from concourse import bacc
from concourse.bass_utils import run_bass_kernel_spmd

F32 = mybir.dt.float32
F32R = mybir.dt.float32r
BF16 = mybir.dt.bfloat16
EXP = mybir.ActivationFunctionType.Exp
ADD = mybir.AluOpType.add
MULT = mybir.AluOpType.mult

CDT = F32R                         # x / W / V / E / att operand dtype
CDT_NP = ml_dtypes.bfloat16 if CDT == BF16 else np.float32
QKDT = F32R                        # Q^T/K^T (scores operands) dtype

B, N, DIM = 4, 2048, 512
DL = 256                          # 4 heads * 64 dims per core
SCALE = 64 ** -0.5
TT = N // 512                     # 4 t-tiles of 512
TC = N // 128                     # 16 t-chunks of 128


def _build(nc, s_pad):
    n_sc = s_pad // 128

    xt = nc.dram_tensor("XT", [DIM, N], CDT, kind="ExternalInput")
    xgt = nc.dram_tensor("XGT", [DIM, s_pad], CDT, kind="ExternalInput")
    ind4 = nc.dram_tensor("IND4", [s_pad, 4], CDT, kind="ExternalInput")
    wqt = nc.dram_tensor("WQT", [DIM, DL], CDT, kind="ExternalInput")
    wkt = nc.dram_tensor("WKT", [DIM, DL], CDT, kind="ExternalInput")
    wvt = nc.dram_tensor("WVT", [DIM, DL], CDT, kind="ExternalInput")
    wot = nc.dram_tensor("WOT", [DL, DIM], CDT, kind="ExternalInput")
    corr = nc.dram_tensor("CORR", [65, 4], F32, kind="ExternalInput")
    outs = [nc.dram_tensor(f"OUT{hp}", [N, DIM], F32, kind="ExternalOutput")
            for hp in range(2)]

    with tile.TileContext(nc) as tc:
        with (
            tc.tile_pool(name="persist", bufs=1) as pp,
            tc.tile_pool(name="xpool", bufs=4) as xp,
            tc.tile_pool(name="ps512", bufs=4, space="PSUM") as ps512,
            tc.tile_pool(name="psreg", bufs=2, space="PSUM") as psreg,
            tc.tile_pool(name="epool", bufs=3) as ep,
            tc.tile_pool(name="npool", bufs=3) as np_pool,
            tc.tile_pool(name="dpool", bufs=2) as dpool,
            tc.tile_pool(name="drampool", bufs=4, space="DRAM") as drp,
            tc.tile_pool(name="ahpool", bufs=2) as ahp,
            tc.tile_pool(name="opool", bufs=4) as op,
        ):
            wq_sb = pp.tile([128, 4 * DL], CDT)
            wk_sb = pp.tile([128, 4 * DL], CDT)
            wv_sb = pp.tile([128, 4 * DL], CDT)
            wo_sb = pp.tile([128, 2 * DIM], CDT)
            corr_sb = pp.tile([65, 4], F32)
            qt_sb = pp.tile([128, 2 * N], QKDT)          # [d-chunk 2][t]
            kt_sb = pp.tile([128, 2 * s_pad], QKDT)      # [d-chunk 2][s]
            v_sb = pp.tile([128, n_sc * 4 * 65], CDT)    # [sc][h][65]
            att_pair = [pp.tile([128, N], CDT, name=f"attp{i}") for i in range(2)]

            # --- input DMAs, critical-path first (wk+xg gate the first scores) ---
            for k in range(4):
                nc.sync.dma_start(wk_sb[:, k * DL:(k + 1) * DL], wkt.ap()[k * 128:(k + 1) * 128, :])
            xg_tiles = []
            for k in range(4):
                xg = xp.tile([128, s_pad], CDT, tag="xg")
                nc.sync.dma_start(xg[:], xgt.ap()[k * 128:(k + 1) * 128, :])
                xg_tiles.append(xg)
            for k in range(4):
                nc.sync.dma_start(wq_sb[:, k * DL:(k + 1) * DL], wqt.ap()[k * 128:(k + 1) * 128, :])
            xt_tiles = [xp.tile([128, N], CDT, tag="xf", name=f"xf{k}") for k in range(4)]
            for k in range(4):
                nc.sync.dma_start(xt_tiles[k][:, 0:512], xt.ap()[k * 128:(k + 1) * 128, 0:512])
            for k in range(4):
                nc.sync.dma_start(wv_sb[:, k * DL:(k + 1) * DL], wvt.ap()[k * 128:(k + 1) * 128, :])
            nc.sync.dma_start(corr_sb[:], corr.ap())
            v_view = v_sb[:].rearrange("p (s h x) -> p s h x", s=n_sc, h=4)
            for sc in range(n_sc):
                nc.sync.dma_start(v_view[:, sc, :, 64], ind4.ap()[sc * 128:(sc + 1) * 128, :])
            for t in range(1, TT):
                for k in range(4):
                    nc.sync.dma_start(xt_tiles[k][:, t * 512:(t + 1) * 512],
                                      xt.ap()[k * 128:(k + 1) * 128, t * 512:(t + 1) * 512])
            for k in range(2):
                nc.sync.dma_start(wo_sb[:, k * DIM:(k + 1) * DIM], wot.ap()[k * 128:(k + 1) * 128, :])

            s_tiles = [(i * 512, min(512, s_pad - i * 512)) for i in range((s_pad + 511) // 512)]

            def emit_kt(dc, s0, sw):
                pk = ps512.tile([128, 512], F32, tag="ps512", name="pk")
                for k in range(4):
                    nc.tensor.matmul(
                        pk[:, :sw],
                        wk_sb[:, k * DL + dc * 128: k * DL + (dc + 1) * 128],
                        xg_tiles[k][:, s0:s0 + sw],
                        start=(k == 0), stop=(k == 3),
                    )
                nc.vector.tensor_copy(kt_sb[:, dc * s_pad + s0: dc * s_pad + s0 + sw], pk[:, :sw])

            def emit_qt(dc, t):
                pq = ps512.tile([128, 512], F32, tag="ps512", name="pq")
                for k in range(4):
                    nc.tensor.matmul(
                        pq[:],
                        wq_sb[:, k * DL + dc * 128: k * DL + (dc + 1) * 128],
                        xt_tiles[k][:, t * 512:(t + 1) * 512],
                        start=(k == 0), stop=(k == 3),
                    )
                nc.vector.tensor_copy(qt_sb[:, dc * N + t * 512: dc * N + (t + 1) * 512], pq[:])

            def emit_v(sc):
                pv = ps512.tile([128, 256], F32, tag="ps512", name="pv")
                for k in range(4):
                    nc.tensor.matmul(
                        pv[:],
                        xg_tiles[k][:, sc * 128:(sc + 1) * 128],
                        wv_sb[:, k * DL:(k + 1) * DL],
                        start=(k == 0), stop=(k == 3),
                    )
                nc.vector.tensor_copy(
                    v_view[:, sc, :, 0:64],
                    pv[:].rearrange("p (h x) -> p h x", h=4),
                )

            def emit_wout_chunk(hp, t):
                po = ps512.tile([128, 512], F32, tag="ps512", name="po")
                nc.tensor.matmul(
                    po[:],
                    att_pair[hp][:, t * 128:(t + 1) * 128],
                    wo_sb[:, hp * DIM:(hp + 1) * DIM],
                    start=True, stop=True,
                )
                o_sb = op.tile([128, 512], F32, tag="o")
                nc.scalar.copy(o_sb[:], po[:])
                nc.sync.dma_start(outs[hp].ap()[t * 128:(t + 1) * 128, :], o_sb[:])

            def emit_normalize_half(hp, h, half, numer_sb):
                # one t-half (1024 tokens) of head h: denominator -> recip -> scale
                sl = slice(half * 1024, (half + 1) * 1024)
                den = dpool.tile([65, 1024], F32, tag="den")
                nc.vector.tensor_scalar_add(
                    den[64:65, :], numer_sb[h][64:65, sl], corr_sb[64:65, h:h + 1])
                scratch = drp.tile([1024], F32, tag="scr")
                nc.sync.dma_start(scratch[:].unsqueeze(0), den[64:65, :])
                bden = dpool.tile([64, 1024], F32, tag="bden")
                nc.sync.dma_start(bden[:], scratch[:].unsqueeze(0).broadcast_to([64, 1024]))
                rbc = dpool.tile([64, 1024], F32, tag="rbc")
                nc.vector.reciprocal_approx_fast(out=rbc[:], in_=bden[:])
                att_h = ahp.tile([64, 1024], CDT, tag="att")
                nc.vector.scalar_tensor_tensor(
                    out=att_h[:], in0=numer_sb[h][0:64, sl],
                    scalar=corr_sb[0:64, h:h + 1], in1=rbc[:],
                    op0=ADD, op1=MULT,
                )
                par = (h % 2) * 64
                nc.sync.dma_start(att_pair[hp][par:par + 64, sl], att_h[:])

            # filler work drip-fed one unit per score/exp group
            fillers = []

            def drain_filler():
                if fillers:
                    fillers.pop(0)()

            # K^T d-chunk 0 gates the whole pipeline: emit first
            for s0, sw in s_tiles:
                emit_kt(0, s0, sw)

            for hp in range(2):
                numer_sb = {}
                for h in (2 * hp, 2 * hp + 1):
                    numer_sb[h] = np_pool.tile([65, N], F32, tag="numer", name=f"numer{h}")
                if hp == 1:
                    # second pair: all QKV done; fillers drain Wout of pair 0
                    fillers.extend([
                        (lambda t=t: emit_wout_chunk(0, t)) for t in range(TC)
                    ])
                for t in range(TT):
                    if hp == 0:
                        emit_qt(0, t)
                        if t == 1:
                            # drip KT dc1 then QT dc1 between upcoming groups
                            for s0, sw in s_tiles:
                                fillers.append(lambda s0=s0, sw=sw: emit_kt(1, s0, sw))
                            for tq in range(TT):
                                fillers.append(lambda tq=tq: emit_qt(1, tq))
                    pn = {}
                    for h in (2 * hp, 2 * hp + 1):
                        pn[h] = ps512.tile([65, 512], F32, tag="ps512", name=f"pn{h}")
                    for sc in range(n_sc):
                        reg = psreg.tile([128, 1024], F32, tag="reg")
                        e_sb = ep.tile([128, 1024], CDT, tag="e")
                        for j, h in enumerate((2 * hp, 2 * hp + 1)):
                            par = (h % 2) * 64
                            nc.tensor.matmul(
                                reg[:, j * 512:(j + 1) * 512],
                                kt_sb[par:par + 64, hp * s_pad + sc * 128: hp * s_pad + (sc + 1) * 128],
                                qt_sb[par:par + 64, hp * N + t * 512: hp * N + (t + 1) * 512],
                                start=True, stop=True,
                            )
                        if hp == 0 and t == 0:
                            emit_v(sc)
                        else:
                            drain_filler()
                        nc.scalar.activation(e_sb[:], reg[:], EXP, scale=SCALE)
                        for j, h in enumerate((2 * hp, 2 * hp + 1)):
                            nc.tensor.matmul(
                                pn[h][:],
                                v_sb[:, (sc * 4 + h) * 65:(sc * 4 + h + 1) * 65],
                                e_sb[:, j * 512:(j + 1) * 512],
                                start=(sc == 0), stop=(sc == n_sc - 1),
                            )
                    for h in (2 * hp, 2 * hp + 1):
                        nc.vector.tensor_copy(numer_sb[h][:, t * 512:(t + 1) * 512], pn[h][:])
                    if t == 1:
                        for h in (2 * hp, 2 * hp + 1):
                            emit_normalize_half(hp, h, 0, numer_sb)
                # finish pair: second halves
                for h in (2 * hp, 2 * hp + 1):
                    emit_normalize_half(hp, h, 1, numer_sb)
                while fillers:
                    drain_filler()
            for t in range(TC):
                emit_wout_chunk(1, t)

    nc.compile()
    return nc


def _prep(input_feature, mask, Wq, Wk, Wv, Wout):
    x = np.ascontiguousarray(np.asarray(input_feature, dtype=np.float32))
    m = np.asarray(mask)
    Wq = np.asarray(Wq, dtype=np.float32)
    Wk = np.asarray(Wk, dtype=np.float32)
    Wv = np.asarray(Wv, dtype=np.float32)
    Wout = np.asarray(Wout, dtype=np.float32)

    idxs = [np.flatnonzero(m[b]) for b in range(B)]
    s_pad = max(128, ((max(len(i) for i in idxs) + 127) // 128) * 128)

    def cvt(a):
        return np.ascontiguousarray(a.astype(CDT_NP))

    in_maps = []
    for c in range(8):
        b, g = c // 2, c % 2
        idx = idxs[b]
        cnt = len(idx)
        xg = np.zeros((s_pad, DIM), np.float32)
        xg[:cnt] = x[b][idx]
        ind4 = np.zeros((s_pad, 4), np.float32)
        ind4[:cnt] = 1.0
        xm = x[b][m[b] == 0].sum(axis=0, dtype=np.float32)
        corr = np.zeros((65, 4), np.float32)
        for h in range(4):
            hg = g * 4 + h
            corr[0:64, h] = Wv[hg * 64:(hg + 1) * 64, :] @ xm
            corr[64, h] = np.float32(N - cnt)
        in_maps.append({
            "XT": cvt(x[b].T),
            "XGT": cvt(xg.T),
            "IND4": cvt(ind4),
            "WQT": cvt(Wq[g * DL:(g + 1) * DL, :].T),
            "WKT": cvt(Wk[g * DL:(g + 1) * DL, :].T),
            "WVT": cvt(Wv[g * DL:(g + 1) * DL, :].T),
            "WOT": cvt(Wout[:, g * DL:(g + 1) * DL].T),
            "CORR": corr,
        })
    return in_maps, s_pad


def _run(in_maps, s_pad, trace=False):
    nc = bacc.Bacc("TRN2", target_bir_lowering=False, debug=False, num_devices=8)
    _build(nc, s_pad)
    res = run_bass_kernel_spmd(nc, in_maps, core_ids=list(range(8)), trace=trace)
    out = np.empty((B, N, DIM), np.float32)
    for b in range(B):
        out[b] = (res.results[2 * b]["OUT0"] + res.results[2 * b]["OUT1"]
                  + res.results[2 * b + 1]["OUT0"] + res.results[2 * b + 1]["OUT1"])
    return out, res


def kernel(input_feature, mask, Wq, Wk, Wv, Wout):
    in_maps, s_pad = _prep(input_feature, mask, Wq, Wk, Wv, Wout)
    out, _ = _run(in_maps, s_pad)
    return out



# revision 15
# speedup vs baseline: 1.0029x; 1.0029x over previous
"""Masked cross-modal attention on 8 Trainium2 NeuronCores (v2).

Reference math (per batch b):
    q,k,v = x @ W{q,k,v}.T   (head-major channels, H=8, Dh=64)
    s     = (q @ k.T) / 8, masked_fill(mask==0, 1e-9), softmax over keys
    out   = (att @ v) @ Wout.T

Masked positions contribute weight exp(1e-9)~=1 and value v_j independent of
the query, so with U = unmasked keys, M = masked keys:
    out[t] = (sum_{j in U} e^{s_tj} v_j + sum_{j in M} v_j)
           / (sum_{j in U} e^{s_tj} + |M|)
The kernel runs attention only over gathered unmasked keys (~half).  The
masked-sum correction is folded into a reserved padding row (index s_pad-1):
its gathered x column is zero so K=0 and the attention weight is exactly
exp(0)=1; its V entry is DMA'd to sum_{j in M} v_j and its denominator
indicator to |M|.  No on-chip correction ops needed.

Sharding: core c -> batch c//2, head-group c%2 (4 of 8 heads).  Each core
emits ONE partial [2048,512] output (both head-pairs accumulated in PSUM
through its Wout slice); the host sums the two partials per batch.

Engine layout per core: PE does QKV projections + scores (bf16) and
exp-weighted value sums / output projection (f32r).  ACT does only exp.
DVE evacuates PSUM (casts), computes the per-token reciprocal and the
normalize multiply (reading pn PSUM directly).  GPSIMD broadcasts the
reciprocal row across partitions, copies nothing from PSUM (no port), and
issues the output DMAs.  PE is pre-warmed with dummy matmuls during the
initial input-DMA stall so HAM unthrottles before real work.
"""

import sys

for _p in ("/opt/trn_rl_repo", "/root/.axon_site/_ro/trn_rl_repo"):
    if _p not in sys.path:
        sys.path.append(_p)

import numpy as np
import ml_dtypes
import concourse.bass as bass
import concourse.mybir as mybir
import concourse.tile as tile
from concourse import bacc
from concourse.bass_utils import run_bass_kernel_spmd

F32 = mybir.dt.float32
F32R = mybir.dt.float32r
BF16 = mybir.dt.bfloat16
EXP = mybir.ActivationFunctionType.Exp

B, N, DIM = 4, 2048, 512
DL = 256                          # 4 heads * 64 dims per core
SCALE = 64 ** -0.5
TT = N // 512                     # 4 t-tiles of 512

# engine/strategy switches for iteration
BCAST_GPSIMD = False              # partition_broadcast vs DRAM-roundtrip DMA
WARMUP_MM = 14                    # dummy matmuls to pre-warm PE / HAM
DEBUG_DUMP = False                # dump KT/QT/V/ATT intermediates


def _build(nc, s_pad):
    n_sc = s_pad // 128

    xt = nc.dram_tensor("XT", [DIM, N], BF16, kind="ExternalInput")
    xgt = nc.dram_tensor("XGT", [DIM, s_pad], BF16, kind="ExternalInput")
    ind4 = nc.dram_tensor("IND4", [s_pad, 4 * 64], F32R, kind="ExternalInput")
    wqt = nc.dram_tensor("WQT", [DIM, DL], BF16, kind="ExternalInput")
    wkt = nc.dram_tensor("WKT", [DIM, DL], BF16, kind="ExternalInput")
    wvt = nc.dram_tensor("WVT", [DIM, DL], BF16, kind="ExternalInput")
    wot = nc.dram_tensor("WOT", [DL, DIM], F32R, kind="ExternalInput")
    corrv = nc.dram_tensor("CORRV", [1, DL], F32R, kind="ExternalInput")
    out_t = nc.dram_tensor("OUT", [N, DIM], F32, kind="ExternalOutput")

    with tile.TileContext(nc) as tc:
        with (
            tc.tile_pool(name="persist", bufs=1) as pp,
            tc.tile_pool(name="psA", bufs=2, space="PSUM") as psA,
            tc.tile_pool(name="psB", bufs=4, space="PSUM") as psB,
            tc.tile_pool(name="epool", bufs=3) as ep,
            tc.tile_pool(name="seedp", bufs=3) as sp_pool,
            tc.tile_pool(name="rbcp", bufs=3) as rp,
            tc.tile_pool(name="opool", bufs=3) as op,
            tc.tile_pool(name="drampool", bufs=2, space="DRAM") as drp,
        ):
            wq_sb = pp.tile([128, 4 * DL], BF16)
            wk_sb = pp.tile([128, 4 * DL], BF16)
            wv_sb = pp.tile([128, 4 * DL], BF16)
            wo_sb = pp.tile([128, 2 * DIM], F32R)
            xt_sb = pp.tile([128, 4 * N], BF16)
            xg_sb = pp.tile([128, 4 * s_pad], BF16)
            qt_sb = pp.tile([128, 2 * N], BF16)          # [dc][t]
            kt_sb = pp.tile([128, 2 * s_pad], BF16)      # [dc][s]
            # per (sc, h): [ind, 63 pad, 64 v-dims] — pn row 0 is the
            # denominator (recip needs base 0), rows 64:128 the values
            # (a 64-partition PSUM access must start at partition 0 or 64)
            v_sb = pp.tile([128, n_sc * 4 * 128], F32R)
            att_pair = [pp.tile([128, N], F32R, name=f"attp{i}") for i in range(2)]

            v_view = v_sb[:].rearrange("p (s h x) -> p s h x", s=n_sc, h=4)

            # --- input DMAs, critical-path first (wk+xg gate the first scores)
            for k in range(4):
                nc.sync.dma_start(wk_sb[:, k * DL:(k + 1) * DL],
                                  wkt.ap()[k * 128:(k + 1) * 128, :])

            # --- PE warmup: dummy matmuls on wk (the first DMA to land) keep
            # HAM busy during the input stall, so real matmuls run at 2.4 GHz.
            for i in range(WARMUP_MM):
                wt = psA.tile([128, 1024], F32, tag="reg", name="warm")
                nc.tensor.matmul(wt[:, 0:256], wk_sb[:, 0:128],
                                 wk_sb[:, 0:256], start=True, stop=True)
            s_tiles = [(i * 512, min(512, s_pad - i * 512))
                       for i in range((s_pad + 511) // 512)]
            for s0, sw in s_tiles:
                for k in range(4):
                    nc.sync.dma_start(
                        xg_sb[:, k * s_pad + s0: k * s_pad + s0 + sw],
                        xgt.ap()[k * 128:(k + 1) * 128, s0:s0 + sw])
            for k in range(4):
                nc.sync.dma_start(wq_sb[:, k * DL:(k + 1) * DL],
                                  wqt.ap()[k * 128:(k + 1) * 128, :])
            for k in range(4):
                nc.sync.dma_start(xt_sb[:, k * N: k * N + 512],
                                  xt.ap()[k * 128:(k + 1) * 128, 0:512])
            for k in range(4):
                nc.sync.dma_start(wv_sb[:, k * DL:(k + 1) * DL],
                                  wvt.ap()[k * 128:(k + 1) * 128, :])
            # ind col 0 + zeroed pad cols 1:64 arrive in one DMA per chunk
            for sc in range(n_sc):
                nc.sync.dma_start(
                    v_view[:, sc, :, 0:64],
                    ind4.ap()[sc * 128:(sc + 1) * 128, :].rearrange(
                        "p (h x) -> p h x", h=4))
            for t in range(1, TT):
                for k in range(4):
                    nc.sync.dma_start(
                        xt_sb[:, k * N + t * 512: k * N + (t + 1) * 512],
                        xt.ap()[k * 128:(k + 1) * 128, t * 512:(t + 1) * 512])
            for k in range(2):
                nc.sync.dma_start(wo_sb[:, k * DIM:(k + 1) * DIM],
                                  wot.ap()[k * 128:(k + 1) * 128, :])

            def emit_kt(dc, s0, sw):
                pk = psB.tile([128, 512], F32, tag="psB", name="pk")
                for k in range(4):
                    nc.tensor.matmul(
                        pk[:, :sw],
                        wk_sb[:, k * DL + dc * 128: k * DL + (dc + 1) * 128],
                        xg_sb[:, k * s_pad + s0: k * s_pad + s0 + sw],
                        start=(k == 0), stop=(k == 3),
                    )
                nc.vector.tensor_copy(
                    kt_sb[:, dc * s_pad + s0: dc * s_pad + s0 + sw], pk[:, :sw])

            def emit_qt(dc, t):
                pq = psB.tile([128, 512], F32, tag="psB", name="pq")
                for k in range(4):
                    nc.tensor.matmul(
                        pq[:],
                        wq_sb[:, k * DL + dc * 128: k * DL + (dc + 1) * 128],
                        xt_sb[:, k * N + t * 512: k * N + (t + 1) * 512],
                        start=(k == 0), stop=(k == 3),
                    )
                nc.vector.tensor_copy(
                    qt_sb[:, dc * N + t * 512: dc * N + (t + 1) * 512], pq[:])

            def emit_v(sc):
                pv = psB.tile([128, 512], F32, tag="psB", name="pv")
                for k in range(4):
                    nc.tensor.matmul(
                        pv[:, 0:256],
                        xg_sb[:, k * s_pad + sc * 128: k * s_pad + (sc + 1) * 128],
                        wv_sb[:, k * DL:(k + 1) * DL],
                        start=(k == 0), stop=(k == 3),
                    )
                nc.vector.tensor_copy(
                    v_view[:, sc, :, 64:128],
                    pv[:, 0:256].rearrange("p (h x) -> p h x", h=4),
                )

            def emit_wout_chunk(c):
                po = psB.tile([128, 512], F32, tag="psB", name="po")
                for hp in range(2):
                    nc.tensor.matmul(
                        po[:],
                        att_pair[hp][:, c * 128:(c + 1) * 128],
                        wo_sb[:, hp * DIM:(hp + 1) * DIM],
                        start=(hp == 0), stop=(hp == 1),
                    )
                o_sb = op.tile([128, 512], F32, tag="o")
                nc.vector.tensor_copy(o_sb[:], po[:])
                nc.gpsimd.dma_start(out_t.ap()[c * 128:(c + 1) * 128, :], o_sb[:])

            def emit_normalize(hp, h, t, pn_tile):
                # one t-tile (512 tokens) of head h, straight from pn PSUM.
                # The denominator lives in pn row 0: custom DVE table ops
                # (reciprocal) only work at partition base 0.
                par = (h % 2) * 64
                seed = sp_pool.tile([128, 512], F32, tag="seed")
                nc.vector.reciprocal_approx_fast(
                    out=seed[0:1, :], in_=pn_tile[0:1, :])
                rbc = rp.tile([64, 512], F32, tag="rbc")
                if BCAST_GPSIMD:
                    nc.gpsimd.partition_broadcast(rbc[:], seed[0:1, :])
                else:
                    scratch = drp.tile([512], F32, tag="scr")
                    nc.sync.dma_start(scratch[:].unsqueeze(0), seed[0:1, :])
                    nc.sync.dma_start(
                        rbc[:], scratch[:].unsqueeze(0).broadcast_to([64, 512]))
                nc.vector.tensor_mul(
                    att_pair[hp][par:par + 64, t * 512:(t + 1) * 512],
                    pn_tile[64:128, :], rbc[:])

            # filler work drip-fed one unit per score/exp group
            fillers = []

            def drain_filler():
                if fillers:
                    fillers.pop(0)()

            # K^T d-chunk 0 gates the whole pipeline: emit first
            for s0, sw in s_tiles:
                emit_kt(0, s0, sw)

            wout_next = 0           # next output chunk to drip
            for hp in range(2):
                for t in range(TT):
                    if hp == 0:
                        emit_qt(0, t)
                        if t == 1:
                            for s0, sw in s_tiles:
                                fillers.append(
                                    lambda s0=s0, sw=sw: emit_kt(1, s0, sw))
                            for tq in range(TT):
                                fillers.append(lambda tq=tq: emit_qt(1, tq))
                    pn = {}
                    for h in (2 * hp, 2 * hp + 1):
                        pn[h] = psB.tile([128, 512], F32, tag="psB",
                                         name=f"pn{h}")
                    for sc in range(n_sc):
                        reg = psA.tile([128, 1024], F32, tag="reg")
                        e_sb = ep.tile([128, 1024], F32R, tag="e")
                        for j, h in enumerate((2 * hp, 2 * hp + 1)):
                            par = j * 64
                            nc.tensor.matmul(
                                reg[:, j * 512:(j + 1) * 512],
                                kt_sb[par:par + 64,
                                      hp * s_pad + sc * 128:
                                      hp * s_pad + (sc + 1) * 128],
                                qt_sb[par:par + 64,
                                      hp * N + t * 512: hp * N + (t + 1) * 512],
                                start=True, stop=True,
                            )
                        if hp == 0 and t == 0:
                            emit_v(sc)
                            if sc == n_sc - 1:
                                # corr row: V(special) = masked-V sum, after
                                # the last emit_v so the DMA lands on top
                                nc.sync.dma_start(
                                    v_view[127:128, n_sc - 1, :, 64:128],
                                    corrv.ap()[:, :].rearrange(
                                        "o (h x) -> o h x", h=4))
                        else:
                            drain_filler()
                            # drip output chunks once both att halves exist:
                            # chunk c needs tokens [c*128,(c+1)*128) of hp1,
                            # normalized after hp1 t-tile (c//4)
                            if hp == 1 and t >= 1 and wout_next < 4 * t:
                                emit_wout_chunk(wout_next)
                                wout_next += 1
                        nc.scalar.activation(e_sb[:], reg[:], EXP, scale=SCALE)
                        for j, h in enumerate((2 * hp, 2 * hp + 1)):
                            nc.tensor.matmul(
                                pn[h][:],
                                v_sb[:, (sc * 4 + h) * 128:(sc * 4 + h + 1) * 128],
                                e_sb[:, j * 512:(j + 1) * 512],
                                start=(sc == 0), stop=(sc == n_sc - 1),
                            )
                    for h in (2 * hp, 2 * hp + 1):
                        emit_normalize(hp, h, t, pn[h])
                while fillers:
                    drain_filler()
            while wout_next < 16:
                emit_wout_chunk(wout_next)
                wout_next += 1

            if DEBUG_DUMP:
                dkt = nc.dram_tensor("DKT", [128, 2 * s_pad], BF16,
                                     kind="ExternalOutput")
                dqt = nc.dram_tensor("DQT", [128, 2 * N], BF16,
                                     kind="ExternalOutput")
                dv = nc.dram_tensor("DV", [128, n_sc * 4 * 128], F32R,
                                    kind="ExternalOutput")
                datt = [nc.dram_tensor(f"DATT{i}", [128, N], F32R,
                                       kind="ExternalOutput")
                        for i in range(2)]
                nc.sync.dma_start(dkt.ap(), kt_sb[:])
                nc.sync.dma_start(dqt.ap(), qt_sb[:])
                nc.sync.dma_start(dv.ap(), v_sb[:])
                for i in range(2):
                    nc.sync.dma_start(datt[i].ap(), att_pair[i][:])

    nc.compile()
    return nc


def _prep(input_feature, mask, Wq, Wk, Wv, Wout):
    x = np.ascontiguousarray(np.asarray(input_feature, dtype=np.float32))
    m = np.asarray(mask)
    Wq = np.asarray(Wq, dtype=np.float32)
    Wk = np.asarray(Wk, dtype=np.float32)
    Wv = np.asarray(Wv, dtype=np.float32)
    Wout = np.asarray(Wout, dtype=np.float32)

    idxs = [np.flatnonzero(m[b]) for b in range(B)]
    # +1 reserves the final padding row for the masked-sum correction
    s_pad = max(128, ((max(len(i) for i in idxs) + 1 + 127) // 128) * 128)

    def bf(a):
        return np.ascontiguousarray(a.astype(ml_dtypes.bfloat16))

    in_maps = []
    for c in range(8):
        b, g = c // 2, c % 2
        idx = idxs[b]
        cnt = len(idx)
        xg = np.zeros((s_pad, DIM), np.float32)
        xg[:cnt] = x[b][idx]
        ind4 = np.zeros((s_pad, 4, 64), np.float32)
        ind4[:cnt, :, 0] = 1.0
        ind4[s_pad - 1, :, 0] = np.float32(N - cnt)
        ind4 = ind4.reshape(s_pad, 256)
        xm = x[b][m[b] == 0].sum(axis=0, dtype=np.float32)
        corrv = (Wv[g * DL:(g + 1) * DL, :] @ xm).reshape(1, DL)
        in_maps.append({
            "XT": bf(x[b].T),
            "XGT": bf(xg.T),
            "IND4": np.ascontiguousarray(ind4),
            "WQT": bf(Wq[g * DL:(g + 1) * DL, :].T),
            "WKT": bf(Wk[g * DL:(g + 1) * DL, :].T),
            "WVT": bf(Wv[g * DL:(g + 1) * DL, :].T),
            "WOT": np.ascontiguousarray(Wout[:, g * DL:(g + 1) * DL].T),
            "CORRV": np.ascontiguousarray(corrv.astype(np.float32)),
        })
    return in_maps, s_pad


def _run(in_maps, s_pad, trace=False):
    nc = bacc.Bacc("TRN2", target_bir_lowering=False, debug=False,
                   num_devices=8)
    _build(nc, s_pad)
    res = run_bass_kernel_spmd(nc, in_maps, core_ids=list(range(8)),
                               trace=trace)
    out = np.empty((B, N, DIM), np.float32)
    for b in range(B):
        out[b] = res.results[2 * b]["OUT"] + res.results[2 * b + 1]["OUT"]
    return out, res


def kernel(input_feature, mask, Wq, Wk, Wv, Wout):
    in_maps, s_pad = _prep(input_feature, mask, Wq, Wk, Wv, Wout)
    out, _ = _run(in_maps, s_pad)
    return out


# revision 18
# speedup vs baseline: 1.0589x; 1.0558x over previous
"""Masked cross-modal attention on 8 Trainium2 NeuronCores (v2).

Reference math (per batch b):
    q,k,v = x @ W{q,k,v}.T   (head-major channels, H=8, Dh=64)
    s     = (q @ k.T) / 8, masked_fill(mask==0, 1e-9), softmax over keys
    out   = (att @ v) @ Wout.T

Masked positions contribute weight exp(1e-9)~=1 and value v_j independent of
the query, so with U = unmasked keys, M = masked keys:
    out[t] = (sum_{j in U} e^{s_tj} v_j + sum_{j in M} v_j)
           / (sum_{j in U} e^{s_tj} + |M|)
The kernel runs attention only over gathered unmasked keys (~half).  The
masked-sum correction is folded into a reserved padding row (index s_pad-1):
its gathered x column is zero so K=0 and the attention weight is exactly
exp(0)=1; its V entry is DMA'd to sum_{j in M} v_j and its denominator
indicator to |M|.  No on-chip correction ops needed.

Sharding: core c -> batch c//2, head-group c%2 (4 of 8 heads).  Each core
emits ONE partial [2048,512] output (both head-pairs accumulated in PSUM
through its Wout slice); the host sums the two partials per batch.

Engine layout per core: PE does QKV projections + scores (bf16) and
exp-weighted value sums / output projection (f32r).  ACT does only exp.
DVE evacuates PSUM (casts), computes the per-token reciprocal and the
normalize multiply (reading pn PSUM directly).  GPSIMD broadcasts the
reciprocal row across partitions, copies nothing from PSUM (no port), and
issues the output DMAs.  PE is pre-warmed with dummy matmuls during the
initial input-DMA stall so HAM unthrottles before real work.
"""

import sys

for _p in ("/opt/trn_rl_repo", "/root/.axon_site/_ro/trn_rl_repo"):
    if _p not in sys.path:
        sys.path.append(_p)

import numpy as np
import ml_dtypes
import concourse.bass as bass
import concourse.mybir as mybir
import concourse.tile as tile
from concourse import bacc
from concourse.bass_utils import run_bass_kernel_spmd

F32 = mybir.dt.float32
F32R = mybir.dt.float32r
BF16 = mybir.dt.bfloat16
EXP = mybir.ActivationFunctionType.Exp

B, N, DIM = 4, 2048, 512
DL = 256                          # 4 heads * 64 dims per core
SCALE = 64 ** -0.5
TT = N // 512                     # 4 t-tiles of 512

# engine/strategy switches for iteration
BCAST_GPSIMD = False              # partition_broadcast vs DRAM-roundtrip DMA
WARMUP_MM = 10                    # dummy matmuls to pre-warm PE / HAM
DEBUG_DUMP = False                # dump KT/QT/V/ATT intermediates


def _build(nc, s_pad):
    n_sc = s_pad // 128

    xt = nc.dram_tensor("XT", [DIM, N], BF16, kind="ExternalInput")
    xgt = nc.dram_tensor("XGT", [DIM, s_pad], BF16, kind="ExternalInput")
    ind4 = nc.dram_tensor("IND4", [s_pad, 4 * 64], BF16, kind="ExternalInput")
    wqt = nc.dram_tensor("WQT", [DIM, DL], BF16, kind="ExternalInput")
    wkt = nc.dram_tensor("WKT", [DIM, DL], BF16, kind="ExternalInput")
    wvt = nc.dram_tensor("WVT", [DIM, DL], BF16, kind="ExternalInput")
    wot = nc.dram_tensor("WOT", [DL, DIM], BF16, kind="ExternalInput")
    corrv = nc.dram_tensor("CORRV", [1, DL], BF16, kind="ExternalInput")
    out_t = nc.dram_tensor("OUT", [N, DIM], F32, kind="ExternalOutput")

    with tile.TileContext(nc) as tc:
        with (
            tc.tile_pool(name="persist", bufs=1) as pp,
            tc.tile_pool(name="psA", bufs=2, space="PSUM") as psA,
            tc.tile_pool(name="psB", bufs=4, space="PSUM") as psB,
            tc.tile_pool(name="epool", bufs=3) as ep,
            tc.tile_pool(name="seedp", bufs=3) as sp_pool,
            tc.tile_pool(name="rbcp", bufs=3) as rp,
            tc.tile_pool(name="opool", bufs=3) as op,
            tc.tile_pool(name="drampool", bufs=2, space="DRAM") as drp,
        ):
            wq_sb = pp.tile([128, 4 * DL], BF16)
            wk_sb = pp.tile([128, 4 * DL], BF16)
            wv_sb = pp.tile([128, 4 * DL], BF16)
            wo_sb = pp.tile([128, 2 * DIM], BF16)
            xt_sb = pp.tile([128, 4 * N], BF16)
            xg_sb = pp.tile([128, 4 * s_pad], BF16)
            qt_sb = pp.tile([128, 2 * N], BF16)          # [dc][t]
            kt_sb = pp.tile([128, 2 * s_pad], BF16)      # [dc][s]
            # per (sc, h): [ind, 63 pad, 64 v-dims] — pn row 0 is the
            # denominator (recip needs base 0), rows 64:128 the values
            # (a 64-partition PSUM access must start at partition 0 or 64)
            v_sb = pp.tile([128, n_sc * 4 * 128], BF16)
            att_pair = [pp.tile([128, N], BF16, name=f"attp{i}") for i in range(2)]

            v_view = v_sb[:].rearrange("p (s h x) -> p s h x", s=n_sc, h=4)

            # --- PE warmup: dummy matmuls on an uninitialized scratch (this
            # path has no race detector; values are irrelevant).  DMA engines
            # take ~8us to start moving data, so without this HAM throttles
            # the first ~15us of real matmuls to 1.2 GHz.
            warm_sb = pp.tile([128, 256], F32)
            nc.vector.memset(warm_sb[:], 0.0)
            for i in range(WARMUP_MM):
                wt = psA.tile([128, 1024], F32, tag="reg", name="warm")
                nc.tensor.matmul(wt[:, 0:256], warm_sb[:, 0:128],
                                 warm_sb[:, 0:256], start=True, stop=True)

            # --- input DMAs, critical-path first (wk+xg gate the first scores)
            for k in range(4):
                nc.sync.dma_start(wk_sb[:, k * DL:(k + 1) * DL],
                                  wkt.ap()[k * 128:(k + 1) * 128, :])
            s_tiles = [(i * 512, min(512, s_pad - i * 512))
                       for i in range((s_pad + 511) // 512)]
            for k in range(4):
                nc.sync.dma_start(xg_sb[:, k * s_pad: (k + 1) * s_pad],
                                  xgt.ap()[k * 128:(k + 1) * 128, :])
            for k in range(4):
                nc.sync.dma_start(wq_sb[:, k * DL:(k + 1) * DL],
                                  wqt.ap()[k * 128:(k + 1) * 128, :])
            for k in range(4):
                nc.sync.dma_start(xt_sb[:, k * N: k * N + 512],
                                  xt.ap()[k * 128:(k + 1) * 128, 0:512])
            # (t0 above split out so qt(dc0,t0) can start early; the rest of
            # X arrives below in three full-width transfers)
            for k in range(4):
                nc.sync.dma_start(wv_sb[:, k * DL:(k + 1) * DL],
                                  wvt.ap()[k * 128:(k + 1) * 128, :])
            # ind col 0 + zeroed pad cols 1:64 arrive in one DMA per chunk
            for sc in range(n_sc):
                nc.sync.dma_start(
                    v_view[:, sc, :, 0:64],
                    ind4.ap()[sc * 128:(sc + 1) * 128, :].rearrange(
                        "p (h x) -> p h x", h=4))
            for k in range(4):
                nc.sync.dma_start(
                    xt_sb[:, k * N + 512: (k + 1) * N],
                    xt.ap()[k * 128:(k + 1) * 128, 512:N])
            for k in range(2):
                nc.sync.dma_start(wo_sb[:, k * DIM:(k + 1) * DIM],
                                  wot.ap()[k * 128:(k + 1) * 128, :])

            def emit_kt(dc, s0, sw):
                pk = psB.tile([128, 512], F32, tag="psB", name="pk")
                for k in range(4):
                    nc.tensor.matmul(
                        pk[:, :sw],
                        wk_sb[:, k * DL + dc * 128: k * DL + (dc + 1) * 128],
                        xg_sb[:, k * s_pad + s0: k * s_pad + s0 + sw],
                        start=(k == 0), stop=(k == 3),
                    )
                nc.vector.tensor_copy(
                    kt_sb[:, dc * s_pad + s0: dc * s_pad + s0 + sw], pk[:, :sw])

            def emit_qt(dc, t):
                pq = psB.tile([128, 512], F32, tag="psB", name="pq")
                for k in range(4):
                    nc.tensor.matmul(
                        pq[:],
                        wq_sb[:, k * DL + dc * 128: k * DL + (dc + 1) * 128],
                        xt_sb[:, k * N + t * 512: k * N + (t + 1) * 512],
                        start=(k == 0), stop=(k == 3),
                    )
                nc.vector.tensor_copy(
                    qt_sb[:, dc * N + t * 512: dc * N + (t + 1) * 512], pq[:])

            def emit_v(sc):
                pv = psB.tile([128, 512], F32, tag="psB", name="pv")
                for k in range(4):
                    nc.tensor.matmul(
                        pv[:, 0:256],
                        xg_sb[:, k * s_pad + sc * 128: k * s_pad + (sc + 1) * 128],
                        wv_sb[:, k * DL:(k + 1) * DL],
                        start=(k == 0), stop=(k == 3),
                    )
                nc.vector.tensor_copy(
                    v_view[:, sc, :, 64:128],
                    pv[:, 0:256].rearrange("p (h x) -> p h x", h=4),
                )

            def emit_wout_chunk(c):
                po = psB.tile([128, 512], F32, tag="psB", name="po")
                for hp in range(2):
                    nc.tensor.matmul(
                        po[:],
                        att_pair[hp][:, c * 128:(c + 1) * 128],
                        wo_sb[:, hp * DIM:(hp + 1) * DIM],
                        start=(hp == 0), stop=(hp == 1),
                    )
                o_sb = op.tile([128, 512], F32, tag="o")
                nc.vector.tensor_copy(o_sb[:], po[:])
                nc.gpsimd.dma_start(out_t.ap()[c * 128:(c + 1) * 128, :], o_sb[:])

            def emit_normalize(hp, h, t, pn_tile):
                # one t-tile (512 tokens) of head h, straight from pn PSUM.
                # The denominator lives in pn row 0: custom DVE table ops
                # (reciprocal) only work at partition base 0.
                par = (h % 2) * 64
                seed = sp_pool.tile([128, 512], F32, tag="seed")
                nc.vector.reciprocal_approx_fast(
                    out=seed[0:1, :], in_=pn_tile[0:1, :])
                rbc = rp.tile([64, 512], F32, tag="rbc")
                if BCAST_GPSIMD:
                    nc.gpsimd.partition_broadcast(rbc[:], seed[0:1, :])
                else:
                    scratch = drp.tile([512], F32, tag="scr")
                    nc.sync.dma_start(scratch[:].unsqueeze(0), seed[0:1, :])
                    nc.sync.dma_start(
                        rbc[:], scratch[:].unsqueeze(0).broadcast_to([64, 512]))
                nc.vector.tensor_mul(
                    att_pair[hp][par:par + 64, t * 512:(t + 1) * 512],
                    pn_tile[64:128, :], rbc[:])

            # filler work drip-fed one unit per score/exp group
            fillers = []

            def drain_filler():
                if fillers:
                    fillers.pop(0)()

            # K^T d-chunk 0 gates the whole pipeline: emit first
            for s0, sw in s_tiles:
                emit_kt(0, s0, sw)

            wout_next = 0           # next output chunk to drip
            for hp in range(2):
                for t in range(TT):
                    if hp == 0:
                        emit_qt(0, t)
                        if t == 1:
                            for s0, sw in s_tiles:
                                fillers.append(
                                    lambda s0=s0, sw=sw: emit_kt(1, s0, sw))
                            for tq in range(TT):
                                fillers.append(lambda tq=tq: emit_qt(1, tq))
                    pn = {}
                    for h in (2 * hp, 2 * hp + 1):
                        pn[h] = psB.tile([128, 512], F32, tag="psB",
                                         name=f"pn{h}")
                    for sc in range(n_sc):
                        reg = psA.tile([128, 1024], F32, tag="reg")
                        e_sb = ep.tile([128, 1024], BF16, tag="e")
                        for j, h in enumerate((2 * hp, 2 * hp + 1)):
                            par = j * 64
                            nc.tensor.matmul(
                                reg[:, j * 512:(j + 1) * 512],
                                kt_sb[par:par + 64,
                                      hp * s_pad + sc * 128:
                                      hp * s_pad + (sc + 1) * 128],
                                qt_sb[par:par + 64,
                                      hp * N + t * 512: hp * N + (t + 1) * 512],
                                start=True, stop=True,
                            )
                        if hp == 0 and t == 0:
                            emit_v(sc)
                            if sc == n_sc - 1:
                                # corr row: V(special) = masked-V sum, after
                                # the last emit_v so the DMA lands on top
                                nc.sync.dma_start(
                                    v_view[127:128, n_sc - 1, :, 64:128],
                                    corrv.ap()[:, :].rearrange(
                                        "o (h x) -> o h x", h=4))
                        else:
                            drain_filler()
                            # drip output chunks once both att halves exist:
                            # chunk c needs tokens [c*128,(c+1)*128) of hp1,
                            # normalized after hp1 t-tile (c//4)
                            cap = 4 * (t - 1) + (4 if sc >= 4 else 0)
                            if hp == 1 and t >= 1 and wout_next < cap:
                                emit_wout_chunk(wout_next)
                                wout_next += 1
                        nc.scalar.activation(e_sb[:], reg[:], EXP, scale=SCALE)
                        for j, h in enumerate((2 * hp, 2 * hp + 1)):
                            nc.tensor.matmul(
                                pn[h][:],
                                v_sb[:, (sc * 4 + h) * 128:(sc * 4 + h + 1) * 128],
                                e_sb[:, j * 512:(j + 1) * 512],
                                start=(sc == 0), stop=(sc == n_sc - 1),
                            )
                    for h in (2 * hp, 2 * hp + 1):
                        emit_normalize(hp, h, t, pn[h])
                while fillers:
                    drain_filler()
            while wout_next < 16:
                emit_wout_chunk(wout_next)
                wout_next += 1

            if DEBUG_DUMP:
                dkt = nc.dram_tensor("DKT", [128, 2 * s_pad], BF16,
                                     kind="ExternalOutput")
                dqt = nc.dram_tensor("DQT", [128, 2 * N], BF16,
                                     kind="ExternalOutput")
                dv = nc.dram_tensor("DV", [128, n_sc * 4 * 128], BF16,
                                    kind="ExternalOutput")
                datt = [nc.dram_tensor(f"DATT{i}", [128, N], BF16,
                                       kind="ExternalOutput")
                        for i in range(2)]
                nc.sync.dma_start(dkt.ap(), kt_sb[:])
                nc.sync.dma_start(dqt.ap(), qt_sb[:])
                nc.sync.dma_start(dv.ap(), v_sb[:])
                for i in range(2):
                    nc.sync.dma_start(datt[i].ap(), att_pair[i][:])

    nc.compile()
    return nc


def _prep(input_feature, mask, Wq, Wk, Wv, Wout):
    x = np.ascontiguousarray(np.asarray(input_feature, dtype=np.float32))
    m = np.asarray(mask)
    Wq = np.asarray(Wq, dtype=np.float32)
    Wk = np.asarray(Wk, dtype=np.float32)
    Wv = np.asarray(Wv, dtype=np.float32)
    Wout = np.asarray(Wout, dtype=np.float32)

    idxs = [np.flatnonzero(m[b]) for b in range(B)]
    # +1 reserves the final padding row for the masked-sum correction
    s_pad = max(128, ((max(len(i) for i in idxs) + 1 + 127) // 128) * 128)

    def bf(a):
        return np.ascontiguousarray(a.astype(ml_dtypes.bfloat16))

    in_maps = []
    for c in range(8):
        b, g = c // 2, c % 2
        idx = idxs[b]
        cnt = len(idx)
        xg = np.zeros((s_pad, DIM), np.float32)
        xg[:cnt] = x[b][idx]
        ind4 = np.zeros((s_pad, 4, 64), np.float32)
        ind4[:cnt, :, 0] = 1.0
        ind4[s_pad - 1, :, 0] = np.float32(N - cnt)
        ind4 = ind4.reshape(s_pad, 256)
        xm = x[b][m[b] == 0].sum(axis=0, dtype=np.float32)
        corrv = (Wv[g * DL:(g + 1) * DL, :] @ xm).reshape(1, DL)
        in_maps.append({
            "XT": bf(x[b].T),
            "XGT": bf(xg.T),
            "IND4": bf(ind4),
            "WQT": bf(Wq[g * DL:(g + 1) * DL, :].T),
            "WKT": bf(Wk[g * DL:(g + 1) * DL, :].T),
            "WVT": bf(Wv[g * DL:(g + 1) * DL, :].T),
            "WOT": bf(Wout[:, g * DL:(g + 1) * DL].T),
            "CORRV": bf(corrv),
        })
    return in_maps, s_pad


def _run(in_maps, s_pad, trace=False):
    nc = bacc.Bacc("TRN2", target_bir_lowering=False, debug=False,
                   num_devices=8)
    _build(nc, s_pad)
    res = run_bass_kernel_spmd(nc, in_maps, core_ids=list(range(8)),
                               trace=trace)
    out = np.empty((B, N, DIM), np.float32)
    for b in range(B):
        out[b] = res.results[2 * b]["OUT"] + res.results[2 * b + 1]["OUT"]
    return out, res


def kernel(input_feature, mask, Wq, Wk, Wv, Wout):
    in_maps, s_pad = _prep(input_feature, mask, Wq, Wk, Wv, Wout)
    out, _ = _run(in_maps, s_pad)
    return out


# revision 19
# speedup vs baseline: 1.2762x; 1.2052x over previous
"""Masked cross-modal attention on 8 Trainium2 NeuronCores (v2).

Reference math (per batch b):
    q,k,v = x @ W{q,k,v}.T   (head-major channels, H=8, Dh=64)
    s     = (q @ k.T) / 8, masked_fill(mask==0, 1e-9), softmax over keys
    out   = (att @ v) @ Wout.T

Masked positions contribute weight exp(1e-9)~=1 and value v_j independent of
the query, so with U = unmasked keys, M = masked keys:
    out[t] = (sum_{j in U} e^{s_tj} v_j + sum_{j in M} v_j)
           / (sum_{j in U} e^{s_tj} + |M|)
The kernel runs attention only over gathered unmasked keys (~half).  The
masked-sum correction is folded into a reserved padding row (index s_pad-1):
its gathered x column is zero so K=0 and the attention weight is exactly
exp(0)=1; its V entry is DMA'd to sum_{j in M} v_j and its denominator
indicator to |M|.  No on-chip correction ops needed.

Sharding: core c -> batch c//2, head-group c%2 (4 of 8 heads).  Each core
emits ONE partial [2048,512] output (both head-pairs accumulated in PSUM
through its Wout slice); the host sums the two partials per batch.

Engine layout per core: PE does QKV projections + scores (bf16) and
exp-weighted value sums / output projection (f32r).  ACT does only exp.
DVE evacuates PSUM (casts), computes the per-token reciprocal and the
normalize multiply (reading pn PSUM directly).  GPSIMD broadcasts the
reciprocal row across partitions, copies nothing from PSUM (no port), and
issues the output DMAs.  PE is pre-warmed with dummy matmuls during the
initial input-DMA stall so HAM unthrottles before real work.
"""

import sys

for _p in ("/opt/trn_rl_repo", "/root/.axon_site/_ro/trn_rl_repo"):
    if _p not in sys.path:
        sys.path.append(_p)

import numpy as np
import ml_dtypes
import concourse.bass as bass
import concourse.mybir as mybir
import concourse.tile as tile
from concourse import bacc
from concourse.bass_utils import run_bass_kernel_spmd

F32 = mybir.dt.float32
F32R = mybir.dt.float32r
BF16 = mybir.dt.bfloat16
EXP = mybir.ActivationFunctionType.Exp

B, N, DIM = 4, 2048, 512
DL = 256                          # 4 heads * 64 dims per core
SCALE = 64 ** -0.5
TT = N // 512                     # 4 t-tiles of 512

# engine/strategy switches for iteration
BCAST_GPSIMD = True              # partition_broadcast vs DRAM-roundtrip DMA
WARMUP_MM = 10                    # dummy matmuls to pre-warm PE / HAM
DEBUG_DUMP = False                # dump KT/QT/V/ATT intermediates


def _build(nc, s_pad):
    n_sc = s_pad // 128

    xt = nc.dram_tensor("XT", [DIM, N], BF16, kind="ExternalInput")
    xgt = nc.dram_tensor("XGT", [DIM, s_pad], BF16, kind="ExternalInput")
    ind4 = nc.dram_tensor("IND4", [s_pad, 4 * 64], F32R, kind="ExternalInput")
    wqt = nc.dram_tensor("WQT", [DIM, DL], BF16, kind="ExternalInput")
    wkt = nc.dram_tensor("WKT", [DIM, DL], BF16, kind="ExternalInput")
    wvt = nc.dram_tensor("WVT", [DIM, DL], BF16, kind="ExternalInput")
    wot = nc.dram_tensor("WOT", [DL, DIM], BF16, kind="ExternalInput")
    corrv = nc.dram_tensor("CORRV", [1, DL], F32R, kind="ExternalInput")
    out_t = nc.dram_tensor("OUT", [N, DIM], F32, kind="ExternalOutput")

    with tile.TileContext(nc) as tc:
        with (
            tc.tile_pool(name="persist", bufs=1) as pp,
            tc.tile_pool(name="psA", bufs=2, space="PSUM") as psA,
            tc.tile_pool(name="psB", bufs=4, space="PSUM") as psB,
            tc.tile_pool(name="epool", bufs=3) as ep,
            tc.tile_pool(name="seedp", bufs=3) as sp_pool,
            tc.tile_pool(name="rbcp", bufs=3) as rp,
            tc.tile_pool(name="opool", bufs=3) as op,
            tc.tile_pool(name="drampool", bufs=2, space="DRAM") as drp,
        ):
            wq_sb = pp.tile([128, 4 * DL], BF16)
            wk_sb = pp.tile([128, 4 * DL], BF16)
            wv_sb = pp.tile([128, 4 * DL], BF16)
            wo_sb = pp.tile([128, 2 * DIM], BF16)
            xt_sb = pp.tile([128, 4 * N], BF16)
            xg_sb = pp.tile([128, 4 * s_pad], BF16)
            qt_sb = pp.tile([128, 2 * N], BF16)          # [dc][t]
            kt_sb = pp.tile([128, 2 * s_pad], BF16)      # [dc][s]
            # per (sc, h): [ind, 63 pad, 64 v-dims] — pn row 0 is the
            # denominator (recip needs base 0), rows 64:128 the values
            # (a 64-partition PSUM access must start at partition 0 or 64)
            v_sb = pp.tile([128, n_sc * 4 * 128], F32R)
            att_pair = [pp.tile([128, N], BF16, name=f"attp{i}") for i in range(2)]

            v_view = v_sb[:].rearrange("p (s h x) -> p s h x", s=n_sc, h=4)

            # --- PE warmup: dummy matmuls on an uninitialized scratch (this
            # path has no race detector; values are irrelevant).  DMA engines
            # take ~8us to start moving data, so without this HAM throttles
            # the first ~15us of real matmuls to 1.2 GHz.
            warm_sb = pp.tile([128, 256], F32)
            nc.vector.memset(warm_sb[:], 0.0)
            for i in range(WARMUP_MM):
                wt = psA.tile([128, 1024], F32, tag="reg", name="warm")
                nc.tensor.matmul(wt[:, 0:256], warm_sb[:, 0:128],
                                 warm_sb[:, 0:256], start=True, stop=True)

            # --- input DMAs, critical-path first (wk+xg gate the first scores)
            for k in range(4):
                nc.sync.dma_start(wk_sb[:, k * DL:(k + 1) * DL],
                                  wkt.ap()[k * 128:(k + 1) * 128, :])
            s_tiles = [(i * 512, min(512, s_pad - i * 512))
                       for i in range((s_pad + 511) // 512)]
            for k in range(4):
                nc.sync.dma_start(xg_sb[:, k * s_pad: (k + 1) * s_pad],
                                  xgt.ap()[k * 128:(k + 1) * 128, :])
            for k in range(4):
                nc.sync.dma_start(wq_sb[:, k * DL:(k + 1) * DL],
                                  wqt.ap()[k * 128:(k + 1) * 128, :])
            for k in range(4):
                nc.sync.dma_start(xt_sb[:, k * N: k * N + 512],
                                  xt.ap()[k * 128:(k + 1) * 128, 0:512])
            # (t0 above split out so qt(dc0,t0) can start early; the rest of
            # X arrives below in three full-width transfers)
            for k in range(4):
                nc.sync.dma_start(wv_sb[:, k * DL:(k + 1) * DL],
                                  wvt.ap()[k * 128:(k + 1) * 128, :])
            # ind col 0 + zeroed pad cols 1:64 arrive in one DMA per chunk
            for sc in range(n_sc):
                nc.sync.dma_start(
                    v_view[:, sc, :, 0:64],
                    ind4.ap()[sc * 128:(sc + 1) * 128, :].rearrange(
                        "p (h x) -> p h x", h=4))
            for k in range(4):
                nc.sync.dma_start(
                    xt_sb[:, k * N + 512: (k + 1) * N],
                    xt.ap()[k * 128:(k + 1) * 128, 512:N])
            for k in range(2):
                nc.sync.dma_start(wo_sb[:, k * DIM:(k + 1) * DIM],
                                  wot.ap()[k * 128:(k + 1) * 128, :])

            def emit_kt(dc, s0, sw):
                pk = psB.tile([128, 512], F32, tag="psB", name="pk")
                for k in range(4):
                    nc.tensor.matmul(
                        pk[:, :sw],
                        wk_sb[:, k * DL + dc * 128: k * DL + (dc + 1) * 128],
                        xg_sb[:, k * s_pad + s0: k * s_pad + s0 + sw],
                        start=(k == 0), stop=(k == 3),
                    )
                nc.vector.tensor_copy(
                    kt_sb[:, dc * s_pad + s0: dc * s_pad + s0 + sw], pk[:, :sw])

            def emit_qt(dc, t):
                pq = psB.tile([128, 512], F32, tag="psB", name="pq")
                for k in range(4):
                    nc.tensor.matmul(
                        pq[:],
                        wq_sb[:, k * DL + dc * 128: k * DL + (dc + 1) * 128],
                        xt_sb[:, k * N + t * 512: k * N + (t + 1) * 512],
                        start=(k == 0), stop=(k == 3),
                    )
                nc.vector.tensor_copy(
                    qt_sb[:, dc * N + t * 512: dc * N + (t + 1) * 512], pq[:])

            def emit_v(sc):
                pv = psB.tile([128, 512], F32, tag="psB", name="pv")
                for k in range(4):
                    nc.tensor.matmul(
                        pv[:, 0:256],
                        xg_sb[:, k * s_pad + sc * 128: k * s_pad + (sc + 1) * 128],
                        wv_sb[:, k * DL:(k + 1) * DL],
                        start=(k == 0), stop=(k == 3),
                    )
                nc.vector.tensor_copy(
                    v_view[:, sc, :, 64:128],
                    pv[:, 0:256].rearrange("p (h x) -> p h x", h=4),
                )

            def emit_wout_chunk(c):
                po = psB.tile([128, 512], F32, tag="psB", name="po")
                for hp in range(2):
                    nc.tensor.matmul(
                        po[:],
                        att_pair[hp][:, c * 128:(c + 1) * 128],
                        wo_sb[:, hp * DIM:(hp + 1) * DIM],
                        start=(hp == 0), stop=(hp == 1),
                    )
                o_sb = op.tile([128, 512], F32, tag="o")
                nc.vector.tensor_copy(o_sb[:], po[:])
                nc.gpsimd.dma_start(out_t.ap()[c * 128:(c + 1) * 128, :], o_sb[:])

            def emit_normalize(hp, h, t, pn_tile):
                # one t-tile (512 tokens) of head h, straight from pn PSUM.
                # The denominator lives in pn row 0: custom DVE table ops
                # (reciprocal) only work at partition base 0.
                par = (h % 2) * 64
                seed = sp_pool.tile([128, 512], F32, tag="seed")
                nc.vector.reciprocal_approx_fast(
                    out=seed[0:1, :], in_=pn_tile[0:1, :])
                rbc = rp.tile([64, 512], F32, tag="rbc")
                if BCAST_GPSIMD:
                    nc.gpsimd.partition_broadcast(rbc[:], seed[0:1, :])
                else:
                    scratch = drp.tile([512], F32, tag="scr")
                    nc.sync.dma_start(scratch[:].unsqueeze(0), seed[0:1, :])
                    nc.sync.dma_start(
                        rbc[:], scratch[:].unsqueeze(0).broadcast_to([64, 512]))
                nc.vector.tensor_mul(
                    att_pair[hp][par:par + 64, t * 512:(t + 1) * 512],
                    pn_tile[64:128, :], rbc[:])

            # filler work drip-fed one unit per score/exp group
            fillers = []

            def drain_filler():
                if fillers:
                    fillers.pop(0)()

            # K^T d-chunk 0 gates the whole pipeline: emit first
            for s0, sw in s_tiles:
                emit_kt(0, s0, sw)

            wout_next = 0           # next output chunk to drip
            for hp in range(2):
                for t in range(TT):
                    if hp == 0:
                        emit_qt(0, t)
                        if t == 1:
                            for s0, sw in s_tiles:
                                fillers.append(
                                    lambda s0=s0, sw=sw: emit_kt(1, s0, sw))
                            for tq in range(TT):
                                fillers.append(lambda tq=tq: emit_qt(1, tq))
                    pn = {}
                    for h in (2 * hp, 2 * hp + 1):
                        pn[h] = psB.tile([128, 512], F32, tag="psB",
                                         name=f"pn{h}")
                    for sc in range(n_sc):
                        reg = psA.tile([128, 1024], F32, tag="reg")
                        e_sb = ep.tile([128, 1024], F32R, tag="e")
                        for j, h in enumerate((2 * hp, 2 * hp + 1)):
                            par = j * 64
                            nc.tensor.matmul(
                                reg[:, j * 512:(j + 1) * 512],
                                kt_sb[par:par + 64,
                                      hp * s_pad + sc * 128:
                                      hp * s_pad + (sc + 1) * 128],
                                qt_sb[par:par + 64,
                                      hp * N + t * 512: hp * N + (t + 1) * 512],
                                start=True, stop=True,
                            )
                        if hp == 0 and t == 0:
                            emit_v(sc)
                            if sc == n_sc - 1:
                                # corr row: V(special) = masked-V sum, after
                                # the last emit_v so the DMA lands on top
                                nc.sync.dma_start(
                                    v_view[127:128, n_sc - 1, :, 64:128],
                                    corrv.ap()[:, :].rearrange(
                                        "o (h x) -> o h x", h=4))
                        else:
                            drain_filler()
                            # drip output chunks once both att halves exist:
                            # chunk c needs tokens [c*128,(c+1)*128) of hp1,
                            # normalized after hp1 t-tile (c//4)
                            cap = 4 * (t - 1) + (4 if sc >= 4 else 0)
                            if hp == 1 and t >= 1 and wout_next < cap:
                                emit_wout_chunk(wout_next)
                                wout_next += 1
                        nc.scalar.activation(e_sb[:], reg[:], EXP, scale=SCALE)
                        for j, h in enumerate((2 * hp, 2 * hp + 1)):
                            nc.tensor.matmul(
                                pn[h][:],
                                v_sb[:, (sc * 4 + h) * 128:(sc * 4 + h + 1) * 128],
                                e_sb[:, j * 512:(j + 1) * 512],
                                start=(sc == 0), stop=(sc == n_sc - 1),
                            )
                    for h in (2 * hp, 2 * hp + 1):
                        emit_normalize(hp, h, t, pn[h])
                while fillers:
                    drain_filler()
            while wout_next < 16:
                emit_wout_chunk(wout_next)
                wout_next += 1

            if DEBUG_DUMP:
                dkt = nc.dram_tensor("DKT", [128, 2 * s_pad], BF16,
                                     kind="ExternalOutput")
                dqt = nc.dram_tensor("DQT", [128, 2 * N], BF16,
                                     kind="ExternalOutput")
                dv = nc.dram_tensor("DV", [128, n_sc * 4 * 128], F32R,
                                    kind="ExternalOutput")
                datt = [nc.dram_tensor(f"DATT{i}", [128, N], BF16,
                                       kind="ExternalOutput")
                        for i in range(2)]
                nc.sync.dma_start(dkt.ap(), kt_sb[:])
                nc.sync.dma_start(dqt.ap(), qt_sb[:])
                nc.sync.dma_start(dv.ap(), v_sb[:])
                for i in range(2):
                    nc.sync.dma_start(datt[i].ap(), att_pair[i][:])

    nc.compile()
    return nc


def _prep(input_feature, mask, Wq, Wk, Wv, Wout):
    x = np.ascontiguousarray(np.asarray(input_feature, dtype=np.float32))
    m = np.asarray(mask)
    Wq = np.asarray(Wq, dtype=np.float32)
    Wk = np.asarray(Wk, dtype=np.float32)
    Wv = np.asarray(Wv, dtype=np.float32)
    Wout = np.asarray(Wout, dtype=np.float32)

    idxs = [np.flatnonzero(m[b]) for b in range(B)]
    # +1 reserves the final padding row for the masked-sum correction
    s_pad = max(128, ((max(len(i) for i in idxs) + 1 + 127) // 128) * 128)

    def bf(a):
        return np.ascontiguousarray(a.astype(ml_dtypes.bfloat16))

    in_maps = []
    for c in range(8):
        b, g = c // 2, c % 2
        idx = idxs[b]
        cnt = len(idx)
        xg = np.zeros((s_pad, DIM), np.float32)
        xg[:cnt] = x[b][idx]
        ind4 = np.zeros((s_pad, 4, 64), np.float32)
        ind4[:cnt, :, 0] = 1.0
        ind4[s_pad - 1, :, 0] = np.float32(N - cnt)
        ind4 = ind4.reshape(s_pad, 256)
        xm = x[b][m[b] == 0].sum(axis=0, dtype=np.float32)
        corrv = (Wv[g * DL:(g + 1) * DL, :] @ xm).reshape(1, DL)
        in_maps.append({
            "XT": bf(x[b].T),
            "XGT": bf(xg.T),
            "IND4": np.ascontiguousarray(ind4),
            "WQT": bf(Wq[g * DL:(g + 1) * DL, :].T),
            "WKT": bf(Wk[g * DL:(g + 1) * DL, :].T),
            "WVT": bf(Wv[g * DL:(g + 1) * DL, :].T),
            "WOT": bf(Wout[:, g * DL:(g + 1) * DL].T),
            "CORRV": np.ascontiguousarray(corrv.astype(np.float32)),
        })
    return in_maps, s_pad


def _run(in_maps, s_pad, trace=False):
    nc = bacc.Bacc("TRN2", target_bir_lowering=False, debug=False,
                   num_devices=8)
    _build(nc, s_pad)
    res = run_bass_kernel_spmd(nc, in_maps, core_ids=list(range(8)),
                               trace=trace)
    out = np.empty((B, N, DIM), np.float32)
    for b in range(B):
        out[b] = res.results[2 * b]["OUT"] + res.results[2 * b + 1]["OUT"]
    return out, res


def kernel(input_feature, mask, Wq, Wk, Wv, Wout):
    in_maps, s_pad = _prep(input_feature, mask, Wq, Wk, Wv, Wout)
    out, _ = _run(in_maps, s_pad)
    return out
